# revision 52
# baseline (speedup 1.0000x reference)
"""LoRA-MHSA Trainium2 kernel (v2).

Data-parallel over batch B=8 (one sample per NeuronCore). The per-sample
LoRA adapters are folded into the weights on the host (W_eff = W +
(alpha/r) * B[sid] @ A[sid]), so the device kernel is a pure 16-head MHSA
with per-core weights:

  qkv = x @ Wqkv_eff.T + b ; SDPA (T=1024, dh=64) ; out = y @ Wp_eff.T + b_p

All PE-path tensors are bf16 (fp32 PSUM accumulation). bf16 gives separate
LDWEIGHTS with FWL + pull-ahead (f32r matmuls self-load their stationary,
serializing ~200ns per matmul), halves DMA/SBUF traffic, and measures
4.4e-3 rel err vs the fp32 reference (gate is 2e-2).

Layout: x and q/k channel-major [C, T]; the two score matmuls of a head
pair sit at SBUF partitions 0-63 / 64-127 so they run concurrently on
disjoint PE subarray row-halves. v natural [T, C] with a 65th ones column
per head (memset once), so PV emits the softmax denominator for free in
PSUM row 64. Exp runs 1024-wide from 2-bank PSUM score groups. Softmax
reciprocals are batched [2, 512] reciprocal_approx_fast instead of
single-lane [1, 512] RECIPROCALs (3.3us each in the old version).

Schedule: qk-build of pair p+1 and v-slab builds interleave with attention
of pair p step-by-step so the PE queue never drains (the HAM clock-gate
keeps the PE at 2.4 GHz only while it stays busy); the output projection
interleaves with the last pair's attention drain.
"""

import sys
from collections import deque

sys.path.insert(0, "/opt/trn_rl_repo")

import numpy as np
import concourse.bass as bass
import concourse.tile as tile
from concourse import bacc, mybir
from concourse.bass_utils import run_bass_kernel_spmd

T = 1024
C = 1024
H = 16
DH = 64
RANK = 8
ALPHA_OVER_RANK = 1.0 / 8.0
SM_SCALE = 0.125  # 1/sqrt(dh)
NCORES = 8

F32 = mybir.dt.float32
BF16 = mybir.dt.bfloat16
EXP = mybir.ActivationFunctionType.Exp
ADD = mybir.AluOpType.add
MULT = mybir.AluOpType.mult

ts = bass.ts

TT = T // 128     # 8 t tiles
CINT = C // 128   # 8 contraction tiles
NPAIR = H // 2    # 8 head pairs


def _build():
    nc = bacc.Bacc("TRN2", target_bir_lowering=False, debug=False)

    xT_d = nc.dram_tensor("xT", [C, T], BF16, kind="ExternalInput")
    wqkT_d = nc.dram_tensor("wqkT", [C, 2048], BF16, kind="ExternalInput")
    wvT_d = nc.dram_tensor("wvT", [C, C], BF16, kind="ExternalInput")
    wpT_d = nc.dram_tensor("wpT", [C, C], BF16, kind="ExternalInput")
    bqk_d = nc.dram_tensor("bqk", [128, 16], F32, kind="ExternalInput")
    bv_d = nc.dram_tensor("bv", [1, C], F32, kind="ExternalInput")
    out_d = nc.dram_tensor("out", [T, C], F32, kind="ExternalOutput")

    with tile.TileContext(nc) as tc:
      with tc.tile_pool(name="res", bufs=1) as res:
        xT = res.tile([128, CINT, T], BF16, tag="xT")
        vv = res.tile([128, TT, H, DH + 1], BF16, tag="vv")
        yt = res.tile([128, NPAIR, T], BF16, tag="yt")
        wv = res.tile([128, CINT, C], BF16, tag="wv")
        bqk = res.tile([128, 16], F32, tag="bqk")
        bvb = res.tile([128, C], F32, tag="bvb")
        brow = res.tile([1, 1, C], F32, tag="brow")

        with tc.tile_pool(name="qkp", bufs=2) as qkp, \
             tc.tile_pool(name="wqp", bufs=3) as wqp, \
             tc.tile_pool(name="esp", bufs=5) as esp, \
             tc.tile_pool(name="nrm", bufs=2) as nrm, \
             tc.tile_pool(name="yup", bufs=4) as yup:

            qktiles = {}
            pv_fifo = deque()

            def dma_wqt(wqt, hp, part, eng=None):
                col0 = part * 1024 + hp * 128
                (eng or nc.sync).dma_start(
                    out=wqt[:],
                    in_=wqkT_d[:, col0 : col0 + 128]
                    .rearrange("(n p) c -> p n c", p=128),
                )

            def alloc_wqt():
                return wqp.tile([128, CINT, 128], BF16, tag="wq", name="wqt")

            # ---- prologue DMAs, critical-path first: pair-0 weight slabs
            # and the xT chunks gate the first matmul; each DMA instruction
            # streams on its own engine, so many small DMAs run in parallel.
            wqt0 = [alloc_wqt(), alloc_wqt()]
            # first-needed halves of the pair-0 weight slabs land first
            wq0view = wqkT_d[:, 0:128].rearrange("(n p) c -> p n c", p=128)
            wk0view = wqkT_d[:, 1024:1152].rearrange("(n p) c -> p n c", p=128)
            nc.sync.dma_start(out=wqt0[0][:, 0:4, :], in_=wq0view[:, 0:4, :])
            nc.scalar.dma_start(out=wqt0[0][:, 4:8, :], in_=wq0view[:, 4:8, :])
            nc.sync.dma_start(out=wqt0[1][:, 0:4, :], in_=wk0view[:, 0:4, :])
            nc.scalar.dma_start(out=wqt0[1][:, 4:8, :], in_=wk0view[:, 4:8, :])
            xview = xT_d.rearrange("(n p) t -> p n t", p=128)
            for k in range(16):
                eng = nc.sync if k % 2 == 0 else nc.scalar
                ci, h = k // 2, k % 2
                eng.dma_start(out=xT[:, ci : ci + 1, h * 512 : h * 512 + 512],
                              in_=xview[:, ci : ci + 1, h * 512 : h * 512 + 512])
            nc.sync.dma_start(out=bqk[:], in_=bqk_d[:])
            nc.sync.dma_start(out=brow[:, 0, :], in_=bv_d[:])
            # v weights: the cch0 half gates the prologue v-build
            wvview = wvT_d.rearrange("(n p) c -> p n c", p=128)
            for k in range(4):
                eng = nc.sync if k % 2 == 0 else nc.scalar
                cch, cih = k // 2, k % 2
                eng.dma_start(
                    out=wv[:, cih * 4 : cih * 4 + 4, ts(cch, 512)],
                    in_=wvview[:, cih * 4 : cih * 4 + 4, ts(cch, 512)],
                )
            nc.gpsimd.partition_broadcast(bvb[:], brow[:, 0, :])
            nc.vector.memset(vv[:, :, :, DH : DH + 1], 1.0)

            def build_steps(hp, pool, wqts=None):
                """qk build for pair hp: channel-major q/k -> qkt [128, 2, T]."""
                qkt = qkp.tile([128, 2, T], BF16, tag="qkt", name="qkt")
                qktiles[hp] = qkt
                for part in range(2):  # 0: q, 1: k
                    if wqts is not None:
                        wqt = wqts[part]
                    else:
                        wqt = alloc_wqt()

                        def load(wqt=wqt, hp=hp, part=part):
                            dma_wqt(wqt, hp, part)
                        yield load
                    pqs = [None, None]
                    for ci in range(CINT):
                        def step(ci=ci, pqs=pqs, wqt=wqt, pool=pool):
                            if ci == 0:
                                pqs[0] = pool.tile([128, 512], F32, tag="bv",
                                                   name="pq")
                                pqs[1] = pool.tile([128, 512], F32, tag="bv",
                                                   name="pq")
                            for tch in range(2):
                                nc.tensor.matmul(
                                    pqs[tch][:], wqt[:, ci, :],
                                    xT[:, ci, ts(tch, 512)],
                                    start=(ci == 0), stop=(ci == CINT - 1),
                                )
                        yield step

                    def fin(part=part, pqs=pqs, qkt=qkt, hp=hp):
                        ct = part * 8 + hp
                        for tch in range(2):
                            nc.vector.tensor_scalar_add(
                                qkt[:, part, ts(tch, 512)], pqs[tch][:],
                                bqk[:, ct : ct + 1],
                            )
                    yield fin

            def v_steps(cch, pool):
                """v build for heads [cch*8, cch*8+8): natural layout into vv."""
                for tt in range(TT):
                    pv = [None]
                    for cig in range(2):
                        def step(cig=cig, pv=pv, tt=tt, pool=pool):
                            if cig == 0:
                                pv[0] = pool.tile([128, 512], F32, tag="bv",
                                                  name="pv")
                            for ci in range(cig * 4, cig * 4 + 4):
                                nc.tensor.matmul(
                                    pv[0][:], xT[:, ci, ts(tt, 128)],
                                    wv[:, ci, ts(cch, 512)],
                                    start=(ci == 0), stop=(ci == CINT - 1),
                                )
                        yield step

                    def fin(pv=pv, tt=tt, cch=cch):
                        nc.vector.tensor_tensor(
                            vv[:, tt, cch * 8 : cch * 8 + 8, 0:DH],
                            pv[0][:].rearrange("p (h d) -> p h d", d=DH),
                            bvb[:, ts(cch, 512)].rearrange("p (h d) -> p h d", d=DH),
                            ADD,
                        )
                    yield fin

            def att_steps(hp):
                """Attention for pair hp. Per tqc: 8 sc/exp steps, a copies
                step, and a norm step. PV matmuls trail their exp by 2 steps
                via pv_fifo."""
                qkt = qktiles[hp]
                for tqc in range(2):
                    ys = [None, None]
                    es_tiles = {}
                    yus = {}
                    # den rows land on partition 0 (free-dim concat) via DMA;
                    # DVE/gpsimd lanes cannot move data across partitions.
                    dd = nrm.tile([1, 2, 512], F32, tag="dd", name="dd")

                    def sc_step(tkt, tqc=tqc, qkt=qkt, ys=ys, es_tiles=es_tiles,
                                hp=hp):
                        if tkt == 0:
                            ys[0] = ysp.tile([DH + 1, 512], F32, tag="ys",
                                             name="ys")
                            ys[1] = ysp.tile([DH + 1, 512], F32, tag="ys",
                                             name="ys")
                        sc = scp.tile([128, 2, 512], F32, tag="sc", name="sc")
                        for sub in range(2):
                            po = sub * DH
                            nc.tensor.matmul(
                                sc[:, sub, :],
                                qkt[po : po + DH, 1, ts(tkt, 128)],
                                qkt[po : po + DH, 0, ts(tqc, 512)],
                                start=True, stop=True,
                            )
                        es = esp.tile([128, 2, 512], BF16, tag="es", name="es")
                        if tkt < TT - 1:
                            nc.scalar.activation(
                                es[:].rearrange("p a b -> p (a b)"),
                                sc[:].rearrange("p a b -> p (a b)"),
                                EXP, scale=SM_SCALE,
                            )
                        else:
                            # split the last exp so pv(7,sub0) starts sooner
                            for sub in range(2):
                                nc.scalar.activation(
                                    es[:, sub, :], sc[:, sub, :],
                                    EXP, scale=SM_SCALE,
                                )
                        es_tiles[tkt] = es

                        for sub in range(2):
                            def pv_sub(tkt=tkt, sub=sub, ys=ys,
                                       es_tiles=es_tiles, hp=hp):
                                es = es_tiles[tkt]
                                nc.tensor.matmul(
                                    ys[sub][:], vv[:, tkt, 2 * hp + sub, :],
                                    es[:, sub, :],
                                    start=(tkt == 0), stop=(tkt == TT - 1),
                                )
                            pv_fifo.append(pv_sub)

                    for tkt in range(TT):
                        def step(tkt=tkt, sc_step=sc_step):
                            sc_step(tkt)
                            while len(pv_fifo) > 6:
                                pv_fifo.popleft()()
                        yield step

                    def drain1():
                        pv_fifo.popleft()()
                        pv_fifo.popleft()()
                    yield drain1

                    def drain2():
                        pv_fifo.popleft()()
                        pv_fifo.popleft()()
                    yield drain2

                    def drain3():
                        pv_fifo.popleft()()
                    yield drain3

                    def copies(ys=ys, dd=dd, yus=yus):
                        while pv_fifo:
                            pv_fifo.popleft()()
                        # DVE copies are partition-aligned (0->0); the
                        # cross-partition moves (sub1 body to partitions
                        # 64-127, den rows to partition 0) go via SBUF DMA.
                        w65 = [None, None]
                        for sub in range(2):
                            w65[sub] = yup.tile([DH + 1, 512], F32, tag="w65",
                                                name="w65")
                            nc.vector.tensor_copy(w65[sub][:], ys[sub][:])
                            nc.sync.dma_start(out=dd[:, sub, :],
                                              in_=w65[sub][DH : DH + 1, :])
                        yuh = yup.tile([128, 512], F32, tag="yuh", name="yuh")
                        nc.sync.dma_start(out=yuh[DH : 2 * DH, :],
                                          in_=w65[1][0:DH, :])
                        yus[0] = w65[0]
                        yus[1] = yuh
                    yield copies

                    def norm(tqc=tqc, dd=dd, yus=yus, hp=hp):
                        rr = nrm.tile([1, 2, 512], F32, tag="rr", name="rr")
                        nc.vector.reciprocal_approx_fast(
                            rr[:].rearrange("p a b -> p (a b)"),
                            dd[:].rearrange("p a b -> p (a b)"),
                        )
                        srcs = [yus.pop(0), yus.pop(1)]
                        for sub in range(2):
                            rb = nrm.tile([128, 512], F32, tag="rb", name="rb")
                            nc.gpsimd.partition_broadcast(rb[:], rr[:, sub, :])
                            po = sub * DH
                            nc.vector.tensor_tensor(
                                yt[po : po + DH, hp, ts(tqc, 512)],
                                srcs[sub][po : po + DH, :],
                                rb[po : po + DH, :], MULT,
                            )
                    yield norm

            def interleave(spine, aux):
                """Emit spine steps with aux steps distributed, biased toward
                the back of the spine where the PE otherwise waits on ACT."""
                spine = list(spine)
                aux = list(aux)
                na, ns = len(aux), len(spine)
                ai = 0
                for i, s in enumerate(spine):
                    s()
                    target = int(na * (((i + 1) / ns) ** 1.8))
                    while ai < target:
                        aux[ai]()
                        ai += 1
                while ai < na:
                    aux[ai]()
                    ai += 1

            # prologue gets the whole PSUM (attention pools open later), so
            # build-0 part q/k and the v tiles never wait on bank reuse.
            with tc.tile_pool(name="bvp0", bufs=6, space="PSUM") as bvp0:
                # HAM warmup: dummy matmuls keep the PE busy during the xT
                # DMA wait so the real matmuls start at the 2.4 GHz clock
                dm = res.tile([1, 528], BF16, tag="dm")
                nc.vector.memset(dm[:], 0.5)
                dps = bvp0.tile([16, 512], F32, tag="warm", name="warm", bufs=1)
                for _ in range(8):
                    nc.tensor.matmul(dps[:], dm[:, 0:16], dm[:, 16:528],
                                     start=True, stop=True)
                for s in build_steps(0, bvp0, wqts=wqt0):
                    s()
                for s in v_steps(0, bvp0):
                    s()

            _scp_cm = tc.tile_pool(name="scp", bufs=2, space="PSUM")
            _ysp_cm = tc.tile_pool(name="ysp", bufs=2, space="PSUM")
            scp = _scp_cm.__enter__()
            ysp = _ysp_cm.__enter__()

            with tc.tile_pool(name="bvp", bufs=2, space="PSUM") as bvp:
                # pairs 0..6: attention(p) ∥ build(p+1) ∥ v-chunk
                vlist = list(v_steps(1, bvp))
                vchunks = [vlist[0:6], vlist[6:12], vlist[12:18], vlist[18:24]]
                for hp in range(NPAIR - 1):
                    aux = list(build_steps(hp + 1, bvp))
                    if hp < 4:
                        aux = aux + vchunks[hp]
                    interleave(att_steps(hp), aux)

            # pair 7 attention ∥ output projection. b_p is added host-side.
            wp = res.tile([128, CINT, C], BF16, tag="wp")
            otp = res.tile([128, TT, C], F32, tag="otp")
            nc.sync.dma_start(
                out=wp[:], in_=wpT_d.rearrange("(n p) c -> p n c", p=128)
            )

            def proj_steps(tts, pool, act_copy):
                # PSUM->SBUF copies: DVE while attention still owns the
                # scalar engine, scalar engine once the exps have drained.
                for tt in tts:
                    for cch in range(2):
                        po = [None]

                        def mstep(cch=cch, tt=tt, po=po, pool=pool):
                            po[0] = pool.tile([128, 512], F32, tag="po",
                                              name="po")
                            for ci in range(CINT):
                                nc.tensor.matmul(
                                    po[0][:], yt[:, ci, ts(tt, 128)],
                                    wp[:, ci, ts(cch, 512)],
                                    start=(ci == 0), stop=(ci == CINT - 1),
                                )
                        yield mstep

                        def cstep(cch=cch, tt=tt, po=po, act_copy=act_copy):
                            dst = otp[:, tt, ts(cch, 512)]
                            if act_copy:
                                nc.scalar.copy(dst, po[0][:])
                            else:
                                nc.vector.tensor_copy(dst, po[0][:])
                            # split the 256KB store across both HWDGE queues
                            # so the final store doesn't serialize the tail
                            for q in range(2):
                                eng = nc.sync if q == 0 else nc.scalar
                                c0 = cch * 512 + q * 256
                                eng.dma_start(
                                    out=out_d[ts(tt, 128), c0 : c0 + 256],
                                    in_=otp[:, tt, c0 : c0 + 256],
                                )
                        yield cstep

            att7 = list(att_steps(NPAIR - 1))
            assert len(att7) == 26
            with tc.tile_pool(name="prj", bufs=2, space="PSUM") as prj:
                for s in att7[:13]:       # tqc0 + its drains/copies/norm
                    s()
                interleave(att7[13:], proj_steps(range(0, 4), prj, False))
            _ysp_cm.__exit__(None, None, None)
            _scp_cm.__exit__(None, None, None)

        # attention pools closed: the remaining proj tiles get a deep
        # PSUM pool so the matmul groups pipeline without copy-waits.
        with tc.tile_pool(name="prj2", bufs=6, space="PSUM") as prj2:
            for s in proj_steps(range(4, TT), prj2, True):
                s()

    nc.compile()
    return nc


_NC_CACHE = {}


def prepare_in_maps(inputs):
    import ml_dtypes
    bf16 = ml_dtypes.bfloat16

    x = np.asarray(inputs["x"], dtype=np.float32)
    sid = np.asarray(inputs["subject_id"]).astype(np.int64)
    W_qkv = np.asarray(inputs["W_qkv"], dtype=np.float32)
    b_qkv = np.asarray(inputs["b_qkv"], dtype=np.float32)
    A1 = np.asarray(inputs["A1"], dtype=np.float32)
    B1 = np.asarray(inputs["B1"], dtype=np.float32)
    W_p = np.asarray(inputs["W_p"], dtype=np.float32)
    b_p = np.asarray(inputs["b_p"], dtype=np.float32)
    A2 = np.asarray(inputs["A2"], dtype=np.float32)
    B2 = np.asarray(inputs["B2"], dtype=np.float32)

    # per-adapter folded weights, computed once per unique sid
    folded = {}
    for s in set(int(v) for v in sid):
        wq_eff = W_qkv + ALPHA_OVER_RANK * (B1[s] @ A1[s])   # [3C, C]
        wp_eff = W_p + ALPHA_OVER_RANK * (B2[s] @ A2[s])     # [C, C]
        wT = np.ascontiguousarray(wq_eff.T)                  # [C, 3C]
        folded[s] = (
            np.ascontiguousarray(wT[:, :2048]).astype(bf16),
            np.ascontiguousarray(wT[:, 2048:]).astype(bf16),
            np.ascontiguousarray(wp_eff.T).astype(bf16),
        )

    # q/k bias tiles [128, 16]: col ct=part*8+hp covers channels
    # [part*1024 + hp*128, +128)
    bqk = np.empty((128, 16), dtype=np.float32)
    for part in range(2):
        for hp in range(8):
            c0 = part * 1024 + hp * 128
            bqk[:, part * 8 + hp] = b_qkv[c0 : c0 + 128]
    bv = np.ascontiguousarray(b_qkv[2048:].reshape(1, C))

    in_maps = []
    for b in range(NCORES):
        s = int(sid[b])
        wqkT, wvT, wpT = folded[s]
        in_maps.append({
            "xT": np.ascontiguousarray(x[b].T).astype(bf16),
            "wqkT": wqkT,
            "wvT": wvT,
            "wpT": wpT,
            "bqk": bqk,
            "bv": bv,
        })
    return in_maps


def kernel(**inputs):
    if "nc" not in _NC_CACHE:
        _NC_CACHE["nc"] = _build()
    nc = _NC_CACHE["nc"]

    in_maps = prepare_in_maps(inputs)
    res = run_bass_kernel_spmd(nc, in_maps, core_ids=list(range(NCORES)))
    out = np.stack([r["out"] for r in res.results], axis=0)
    # b_p is added here rather than on-device (kept off the drain tail)
    out += np.asarray(inputs["b_p"], dtype=np.float32)[None, None, :]
    return out.astype(np.float32)


# revision 55
# speedup vs baseline: 1.1768x; 1.1768x over previous
"""LoRA-MHSA Trainium2 kernel (v2).

Data-parallel over batch B=8 (one sample per NeuronCore). The per-sample
LoRA adapters are folded into the weights on the host (W_eff = W +
(alpha/r) * B[sid] @ A[sid]), so the device kernel is a pure 16-head MHSA
with per-core weights:

  qkv = x @ Wqkv_eff.T + b ; SDPA (T=1024, dh=64) ; out = y @ Wp_eff.T + b_p

All PE-path tensors are bf16 (fp32 PSUM accumulation). bf16 gives separate
LDWEIGHTS with FWL + pull-ahead (f32r matmuls self-load their stationary,
serializing ~200ns per matmul), halves DMA/SBUF traffic, and measures
4.4e-3 rel err vs the fp32 reference (gate is 2e-2).

Layout: x and q/k channel-major [C, T]; the two score matmuls of a head
pair sit at SBUF partitions 0-63 / 64-127 so they run concurrently on
disjoint PE subarray row-halves. v natural [T, C] with a 65th ones column
per head (memset once), so PV emits the softmax denominator for free in
PSUM row 64. Exp runs 1024-wide from 2-bank PSUM score groups. Softmax
reciprocals are batched [2, 512] reciprocal_approx_fast instead of
single-lane [1, 512] RECIPROCALs (3.3us each in the old version).

Schedule: qk-build of pair p+1 and v-slab builds interleave with attention
of pair p step-by-step so the PE queue never drains (the HAM clock-gate
keeps the PE at 2.4 GHz only while it stays busy); the output projection
interleaves with the last pair's attention drain.
"""

import sys
from collections import deque

sys.path.insert(0, "/opt/trn_rl_repo")

import numpy as np
import concourse.bass as bass
import concourse.tile as tile
from concourse import bacc, mybir
from concourse.bass_utils import run_bass_kernel_spmd

T = 1024
C = 1024
H = 16
DH = 64
RANK = 8
ALPHA_OVER_RANK = 1.0 / 8.0
SM_SCALE = 0.125  # 1/sqrt(dh)
NCORES = 8

F32 = mybir.dt.float32
BF16 = mybir.dt.bfloat16
EXP = mybir.ActivationFunctionType.Exp
ADD = mybir.AluOpType.add
MULT = mybir.AluOpType.mult

ts = bass.ts

TT = T // 128     # 8 t tiles
CINT = C // 128   # 8 contraction tiles
NPAIR = H // 2    # 8 head pairs


def _build():
    nc = bacc.Bacc("TRN2", target_bir_lowering=False, debug=False)

    xT_d = nc.dram_tensor("xT", [C, T], BF16, kind="ExternalInput")
    wqkT_d = nc.dram_tensor("wqkT", [C, 2048], BF16, kind="ExternalInput")
    wvT_d = nc.dram_tensor("wvT", [C, C], BF16, kind="ExternalInput")
    wpT_d = nc.dram_tensor("wpT", [C, C], BF16, kind="ExternalInput")
    bqk_d = nc.dram_tensor("bqk", [128, 16], F32, kind="ExternalInput")
    bv_d = nc.dram_tensor("bv", [1, C], F32, kind="ExternalInput")
    out_d = nc.dram_tensor("out", [T, C], F32, kind="ExternalOutput")

    with tile.TileContext(nc) as tc:
      with tc.tile_pool(name="res", bufs=1) as res:
        xT = res.tile([128, CINT, T], BF16, tag="xT")
        vv = res.tile([128, TT, H, DH + 1], BF16, tag="vv")
        yt = res.tile([128, NPAIR, T], BF16, tag="yt")
        wv = res.tile([128, CINT, C], BF16, tag="wv")
        bqk = res.tile([128, 16], F32, tag="bqk")
        bvb = res.tile([128, C], F32, tag="bvb")
        brow = res.tile([1, 1, C], F32, tag="brow")

        with tc.tile_pool(name="qkp", bufs=2) as qkp, \
             tc.tile_pool(name="wqp", bufs=3) as wqp, \
             tc.tile_pool(name="esp", bufs=6) as esp, \
             tc.tile_pool(name="nrm", bufs=2) as nrm, \
             tc.tile_pool(name="yup", bufs=4) as yup:

            qktiles = {}
            pv_fifo = deque()

            def dma_wqt(wqt, hp, part, eng=None):
                col0 = part * 1024 + hp * 128
                (eng or nc.sync).dma_start(
                    out=wqt[:],
                    in_=wqkT_d[:, col0 : col0 + 128]
                    .rearrange("(n p) c -> p n c", p=128),
                )

            def alloc_wqt():
                return wqp.tile([128, CINT, 128], BF16, tag="wq", name="wqt")

            # ---- prologue DMAs, critical-path first: pair-0 weight slabs
            # and the xT chunks gate the first matmul; each DMA instruction
            # streams on its own engine, so many small DMAs run in parallel.
            wqt0 = [alloc_wqt(), alloc_wqt()]
            # first-needed halves of the pair-0 weight slabs land first
            wq0view = wqkT_d[:, 0:128].rearrange("(n p) c -> p n c", p=128)
            wk0view = wqkT_d[:, 1024:1152].rearrange("(n p) c -> p n c", p=128)
            nc.sync.dma_start(out=wqt0[0][:, 0:4, :], in_=wq0view[:, 0:4, :])
            nc.scalar.dma_start(out=wqt0[0][:, 4:8, :], in_=wq0view[:, 4:8, :])
            nc.sync.dma_start(out=wqt0[1][:, 0:4, :], in_=wk0view[:, 0:4, :])
            nc.scalar.dma_start(out=wqt0[1][:, 4:8, :], in_=wk0view[:, 4:8, :])
            xview = xT_d.rearrange("(n p) t -> p n t", p=128)
            for k in range(16):
                eng = nc.sync if k % 2 == 0 else nc.scalar
                ci, h = k // 2, k % 2
                eng.dma_start(out=xT[:, ci : ci + 1, h * 512 : h * 512 + 512],
                              in_=xview[:, ci : ci + 1, h * 512 : h * 512 + 512])
            nc.sync.dma_start(out=bqk[:], in_=bqk_d[:])
            nc.sync.dma_start(out=brow[:, 0, :], in_=bv_d[:])
            # v weights: the cch0 half gates the prologue v-build
            wvview = wvT_d.rearrange("(n p) c -> p n c", p=128)
            for k in range(4):
                eng = nc.sync if k % 2 == 0 else nc.scalar
                cch, cih = k // 2, k % 2
                eng.dma_start(
                    out=wv[:, cih * 4 : cih * 4 + 4, ts(cch, 512)],
                    in_=wvview[:, cih * 4 : cih * 4 + 4, ts(cch, 512)],
                )
            nc.gpsimd.partition_broadcast(bvb[:], brow[:, 0, :])
            nc.vector.memset(vv[:, :, :, DH : DH + 1], 1.0)

            def build_steps(hp, pool, wqts=None):
                """qk build for pair hp: channel-major q/k -> qkt [128, 2, T]."""
                qkt = qkp.tile([128, 2, T], BF16, tag="qkt", name="qkt")
                qktiles[hp] = qkt
                for part in range(2):  # 0: q, 1: k
                    if wqts is not None:
                        wqt = wqts[part]
                    else:
                        wqt = alloc_wqt()

                        def load(wqt=wqt, hp=hp, part=part):
                            dma_wqt(wqt, hp, part)
                        yield load
                    pqs = [None, None]
                    for ci in range(CINT):
                        def step(ci=ci, pqs=pqs, wqt=wqt, pool=pool):
                            if ci == 0:
                                pqs[0] = pool.tile([128, 512], F32, tag="bv",
                                                   name="pq")
                                pqs[1] = pool.tile([128, 512], F32, tag="bv",
                                                   name="pq")
                            for tch in range(2):
                                nc.tensor.matmul(
                                    pqs[tch][:], wqt[:, ci, :],
                                    xT[:, ci, ts(tch, 512)],
                                    start=(ci == 0), stop=(ci == CINT - 1),
                                )
                        yield step

                    def fin(part=part, pqs=pqs, qkt=qkt, hp=hp):
                        ct = part * 8 + hp
                        for tch in range(2):
                            nc.vector.tensor_scalar_add(
                                qkt[:, part, ts(tch, 512)], pqs[tch][:],
                                bqk[:, ct : ct + 1],
                            )
                    yield fin

            def v_steps(cch, pool):
                """v build for heads [cch*8, cch*8+8): natural layout into vv."""
                for tt in range(TT):
                    pv = [None]
                    for cig in range(2):
                        def step(cig=cig, pv=pv, tt=tt, pool=pool):
                            if cig == 0:
                                pv[0] = pool.tile([128, 512], F32, tag="bv",
                                                  name="pv")
                            for ci in range(cig * 4, cig * 4 + 4):
                                nc.tensor.matmul(
                                    pv[0][:], xT[:, ci, ts(tt, 128)],
                                    wv[:, ci, ts(cch, 512)],
                                    start=(ci == 0), stop=(ci == CINT - 1),
                                )
                        yield step

                    def fin(pv=pv, tt=tt, cch=cch):
                        nc.vector.tensor_tensor(
                            vv[:, tt, cch * 8 : cch * 8 + 8, 0:DH],
                            pv[0][:].rearrange("p (h d) -> p h d", d=DH),
                            bvb[:, ts(cch, 512)].rearrange("p (h d) -> p h d", d=DH),
                            ADD,
                        )
                    yield fin

            def att_steps(hp):
                """Attention for pair hp. Per tqc: 8 sc/exp steps, a copies
                step, and a norm step. PV matmuls trail their exp by 2 steps
                via pv_fifo."""
                qkt = qktiles[hp]
                for tqc in range(2):
                    ys = [None, None]
                    es_tiles = {}
                    yus = {}
                    # den rows land on partition 0 (free-dim concat) via DMA;
                    # DVE/gpsimd lanes cannot move data across partitions.
                    dd = nrm.tile([1, 2, 512], F32, tag="dd", name="dd")

                    def sc_step(tkt, tqc=tqc, qkt=qkt, ys=ys, es_tiles=es_tiles,
                                hp=hp):
                        if tkt == 0:
                            ys[0] = ysp.tile([DH + 1, 512], F32, tag="ys",
                                             name="ys")
                            ys[1] = ysp.tile([DH + 1, 512], F32, tag="ys",
                                             name="ys")
                        sc = scp.tile([128, 2, 512], F32, tag="sc", name="sc")
                        for sub in range(2):
                            po = sub * DH
                            nc.tensor.matmul(
                                sc[:, sub, :],
                                qkt[po : po + DH, 1, ts(tkt, 128)],
                                qkt[po : po + DH, 0, ts(tqc, 512)],
                                start=True, stop=True,
                            )
                        es = esp.tile([128, 2, 512], BF16, tag="es", name="es")
                        if tkt < TT - 1:
                            nc.scalar.activation(
                                es[:].rearrange("p a b -> p (a b)"),
                                sc[:].rearrange("p a b -> p (a b)"),
                                EXP, scale=SM_SCALE,
                            )
                        else:
                            # split the last exp so pv(7,sub0) starts sooner
                            for sub in range(2):
                                nc.scalar.activation(
                                    es[:, sub, :], sc[:, sub, :],
                                    EXP, scale=SM_SCALE,
                                )
                        es_tiles[tkt] = es

                        for sub in range(2):
                            def pv_sub(tkt=tkt, sub=sub, ys=ys,
                                       es_tiles=es_tiles, hp=hp):
                                es = es_tiles[tkt]
                                nc.tensor.matmul(
                                    ys[sub][:], vv[:, tkt, 2 * hp + sub, :],
                                    es[:, sub, :],
                                    start=(tkt == 0), stop=(tkt == TT - 1),
                                )
                            pv_fifo.append(pv_sub)

                    for tkt in range(TT):
                        def step(tkt=tkt, sc_step=sc_step):
                            sc_step(tkt)
                            while len(pv_fifo) > 8:
                                pv_fifo.popleft()()
                        yield step

                    def drain1():
                        pv_fifo.popleft()()
                        pv_fifo.popleft()()
                        pv_fifo.popleft()()
                    yield drain1

                    def drain2():
                        pv_fifo.popleft()()
                        pv_fifo.popleft()()
                        pv_fifo.popleft()()
                    yield drain2

                    def drain3():
                        pv_fifo.popleft()()
                    yield drain3

                    def copies(ys=ys, dd=dd, yus=yus):
                        while pv_fifo:
                            pv_fifo.popleft()()
                        # DVE copies are partition-aligned (0->0); the
                        # cross-partition moves (sub1 body to partitions
                        # 64-127, den rows to partition 0) go via SBUF DMA.
                        w65 = [None, None]
                        for sub in range(2):
                            w65[sub] = yup.tile([DH + 1, 512], F32, tag="w65",
                                                name="w65")
                            nc.vector.tensor_copy(w65[sub][:], ys[sub][:])
                            nc.sync.dma_start(out=dd[:, sub, :],
                                              in_=w65[sub][DH : DH + 1, :])
                        yuh = yup.tile([128, 512], F32, tag="yuh", name="yuh")
                        nc.sync.dma_start(out=yuh[DH : 2 * DH, :],
                                          in_=w65[1][0:DH, :])
                        yus[0] = w65[0]
                        yus[1] = yuh
                    yield copies

                    def norm(tqc=tqc, dd=dd, yus=yus, hp=hp):
                        rr = nrm.tile([1, 2, 512], F32, tag="rr", name="rr")
                        nc.vector.reciprocal_approx_fast(
                            rr[:].rearrange("p a b -> p (a b)"),
                            dd[:].rearrange("p a b -> p (a b)"),
                        )
                        srcs = [yus.pop(0), yus.pop(1)]
                        for sub in range(2):
                            rb = nrm.tile([128, 512], F32, tag="rb", name="rb")
                            nc.gpsimd.partition_broadcast(rb[:], rr[:, sub, :])
                            po = sub * DH
                            nc.vector.tensor_tensor(
                                yt[po : po + DH, hp, ts(tqc, 512)],
                                srcs[sub][po : po + DH, :],
                                rb[po : po + DH, :], MULT,
                            )
                    yield norm

            def interleave(spine, aux):
                """Emit spine steps with aux steps distributed, biased toward
                the back of the spine where the PE otherwise waits on ACT."""
                spine = list(spine)
                aux = list(aux)
                na, ns = len(aux), len(spine)
                ai = 0
                for i, s in enumerate(spine):
                    s()
                    target = int(na * (((i + 1) / ns) ** 1.8))
                    while ai < target:
                        aux[ai]()
                        ai += 1
                while ai < na:
                    aux[ai]()
                    ai += 1

            # prologue gets the whole PSUM (attention pools open later), so
            # build-0 part q/k and the v tiles never wait on bank reuse.
            with tc.tile_pool(name="bvp0", bufs=6, space="PSUM") as bvp0:
                # HAM warmup: dummy matmuls keep the PE busy during the xT
                # DMA wait so the real matmuls start at the 2.4 GHz clock
                dm = res.tile([1, 528], BF16, tag="dm")
                nc.vector.memset(dm[:], 0.5)
                dps = bvp0.tile([16, 512], F32, tag="warm", name="warm", bufs=1)
                for _ in range(8):
                    nc.tensor.matmul(dps[:], dm[:, 0:16], dm[:, 16:528],
                                     start=True, stop=True)
                for s in build_steps(0, bvp0, wqts=wqt0):
                    s()
                for s in v_steps(0, bvp0):
                    s()

            _scp_cm = tc.tile_pool(name="scp", bufs=2, space="PSUM")
            _ysp_cm = tc.tile_pool(name="ysp", bufs=2, space="PSUM")
            scp = _scp_cm.__enter__()
            ysp = _ysp_cm.__enter__()

            with tc.tile_pool(name="bvp", bufs=2, space="PSUM") as bvp:
                # pairs 0..6: attention(p) ∥ build(p+1) ∥ v-chunk
                vlist = list(v_steps(1, bvp))
                vchunks = [vlist[0:6], vlist[6:12], vlist[12:18], vlist[18:24]]
                for hp in range(NPAIR - 1):
                    aux = list(build_steps(hp + 1, bvp))
                    if hp < 4:
                        aux = aux + vchunks[hp]
                    interleave(att_steps(hp), aux)

            # pair 7 attention ∥ output projection. b_p is added host-side.
            wp = res.tile([128, CINT, C], BF16, tag="wp")
            otp = res.tile([128, TT, C], F32, tag="otp")
            nc.sync.dma_start(
                out=wp[:], in_=wpT_d.rearrange("(n p) c -> p n c", p=128)
            )

            def proj_steps(tts, pool, act_copy):
                # PSUM->SBUF copies: DVE while attention still owns the
                # scalar engine, scalar engine once the exps have drained.
                for tt in tts:
                    for cch in range(2):
                        po = [None]

                        def mstep(cch=cch, tt=tt, po=po, pool=pool):
                            po[0] = pool.tile([128, 512], F32, tag="po",
                                              name="po")
                            for ci in range(CINT):
                                nc.tensor.matmul(
                                    po[0][:], yt[:, ci, ts(tt, 128)],
                                    wp[:, ci, ts(cch, 512)],
                                    start=(ci == 0), stop=(ci == CINT - 1),
                                )
                        yield mstep

                        def cstep(cch=cch, tt=tt, po=po, act_copy=act_copy):
                            dst = otp[:, tt, ts(cch, 512)]
                            # drain phase: ACT and DVE are both idle, so the
                            # two copies of a tile run on different engines
                            if act_copy and cch == 0:
                                nc.scalar.copy(dst, po[0][:])
                            else:
                                nc.vector.tensor_copy(dst, po[0][:])
                            # split the 256KB store across both HWDGE queues
                            # so the final store doesn't serialize the tail
                            for q in range(2):
                                eng = nc.sync if q == 0 else nc.scalar
                                c0 = cch * 512 + q * 256
                                eng.dma_start(
                                    out=out_d[ts(tt, 128), c0 : c0 + 256],
                                    in_=otp[:, tt, c0 : c0 + 256],
                                )
                        yield cstep

            att7 = list(att_steps(NPAIR - 1))
            assert len(att7) == 26
            with tc.tile_pool(name="prj", bufs=2, space="PSUM") as prj:
                for s in att7[:13]:       # tqc0 + its drains/copies/norm
                    s()
                interleave(att7[13:], proj_steps(range(0, 4), prj, False))
            _ysp_cm.__exit__(None, None, None)
            _scp_cm.__exit__(None, None, None)

        # attention pools closed: the remaining proj tiles get a deep
        # PSUM pool so the matmul groups pipeline without copy-waits.
        with tc.tile_pool(name="prj2", bufs=6, space="PSUM") as prj2:
            for s in proj_steps(range(4, TT), prj2, True):
                s()

    nc.compile()
    return nc


_NC_CACHE = {}


def prepare_in_maps(inputs):
    import ml_dtypes
    bf16 = ml_dtypes.bfloat16

    x = np.asarray(inputs["x"], dtype=np.float32)
    sid = np.asarray(inputs["subject_id"]).astype(np.int64)
    W_qkv = np.asarray(inputs["W_qkv"], dtype=np.float32)
    b_qkv = np.asarray(inputs["b_qkv"], dtype=np.float32)
    A1 = np.asarray(inputs["A1"], dtype=np.float32)
    B1 = np.asarray(inputs["B1"], dtype=np.float32)
    W_p = np.asarray(inputs["W_p"], dtype=np.float32)
    b_p = np.asarray(inputs["b_p"], dtype=np.float32)
    A2 = np.asarray(inputs["A2"], dtype=np.float32)
    B2 = np.asarray(inputs["B2"], dtype=np.float32)

    # per-adapter folded weights, computed once per unique sid
    folded = {}
    for s in set(int(v) for v in sid):
        wq_eff = W_qkv + ALPHA_OVER_RANK * (B1[s] @ A1[s])   # [3C, C]
        wp_eff = W_p + ALPHA_OVER_RANK * (B2[s] @ A2[s])     # [C, C]
        wT = np.ascontiguousarray(wq_eff.T)                  # [C, 3C]
        folded[s] = (
            np.ascontiguousarray(wT[:, :2048]).astype(bf16),
            np.ascontiguousarray(wT[:, 2048:]).astype(bf16),
            np.ascontiguousarray(wp_eff.T).astype(bf16),
        )

    # q/k bias tiles [128, 16]: col ct=part*8+hp covers channels
    # [part*1024 + hp*128, +128)
    bqk = np.empty((128, 16), dtype=np.float32)
    for part in range(2):
        for hp in range(8):
            c0 = part * 1024 + hp * 128
            bqk[:, part * 8 + hp] = b_qkv[c0 : c0 + 128]
    bv = np.ascontiguousarray(b_qkv[2048:].reshape(1, C))

    in_maps = []
    for b in range(NCORES):
        s = int(sid[b])
        wqkT, wvT, wpT = folded[s]
        in_maps.append({
            "xT": np.ascontiguousarray(x[b].T).astype(bf16),
            "wqkT": wqkT,
            "wvT": wvT,
            "wpT": wpT,
            "bqk": bqk,
            "bv": bv,
        })
    return in_maps


def kernel(**inputs):
    if "nc" not in _NC_CACHE:
        _NC_CACHE["nc"] = _build()
    nc = _NC_CACHE["nc"]

    in_maps = prepare_in_maps(inputs)
    res = run_bass_kernel_spmd(nc, in_maps, core_ids=list(range(NCORES)))
    out = np.stack([r["out"] for r in res.results], axis=0)
    # b_p is added here rather than on-device (kept off the drain tail)
    out += np.asarray(inputs["b_p"], dtype=np.float32)[None, None, :]
    return out.astype(np.float32)
